# revision 17
# baseline (speedup 1.0000x reference)
"""Trainium2 Bass kernel: batch-sharded fused attention (nn_Attention_48893907698232).

Reference computation (per batch b):
    q = x @ wq.T + bq ; k = x @ wk.T + bk ; v = x @ wv.T + bv
    q, k <- fixed RoPE rotation (same rotation at every position)
    out  = softmax((q @ k.T) / sqrt(512)) @ v

Strategy (v1 -> v2 changes marked *):
  * Data-parallel over batch: 16 batches -> 2 per NeuronCore, no collectives.
  * RoPE folded into projection weights/biases on the host.
  * Host pre-transposes/pre-tiles x and weights so every matmul contraction
    dim lands on SBUF partitions with zero on-chip transposes:
      qT[o,s]  = sum_i wqT[i,o] * xT[i,s]     (PSUM: o on partitions)
      kT[o,s]  likewise
      v[s,o]   = sum_i xT[i,s] * wvT[i,o]     (PSUM: s on partitions)
      ST[sk,sq]= sum_d kT[d,sk] * qT[d,sq]    -> exp(scale*ST) on ACT -> ET
      out[sq,:]= sum_sk ET[sk,sq] * [vA|1|vB|1]  (ones columns => softmax
                                                  rowsum in cols 128/257)
      out  <- out[:, 0:128 & 129:257] * (1 / rowsum)
  * Softmax runs without max-subtraction (scores ~N(0,0.75), exp safe).
  * Matmul operands bf16 (fp32 PSUM accumulate).
* Startup: wq/wk split into 128-col halves so the first matmul's weight
    dependency is 128KB, first-needed tiles are FIFO-ordered across the
    three DMA trigger queues (sync/scalar HWDGE + gpsimd SWDGE), and the
    first two q-units' contractions are interleaved to match DMA arrival.
* PE warm-up junk matmuls sized to hand off to real work with no gap
    (a PE-activity gap appears to reset the HAM clock-ramp timer).
* v is stored as [vA|1|vB|1] so the final AV unit can be column-split
    into two half-width PSUM accumulations: the first half's epilogue +
    output DMA overlap the second half's matmuls, shrinking the tail.
"""

import math
import os
import sys

import numpy as np

os.environ.setdefault("MYCRO_LOCAL_CACHE", "1")
if "/opt/trn_rl_repo" not in sys.path:
    sys.path.insert(0, "/opt/trn_rl_repo")

from contextlib import ExitStack

import concourse.bass as bass
import concourse.tile as tile
from concourse import bacc, mybir
from concourse.bass_utils import run_bass_kernel_spmd

B, S, IN_DIM, OUT_DIM = 16, 1024, 512, 256
THETA = 10000.0
N_CORES = 8
B_LOC = B // N_CORES          # batches per core
I_T = IN_DIM // 128           # 4 contraction tiles for the projections
O_T = OUT_DIM // 128          # 2 feature tiles
S_TILES = S // 128            # 8 sequence tiles
SC = S // 512                 # 2 sequence chunks of 512
SCALE = 1.0 / math.sqrt(IN_DIM)
VW = OUT_DIM + 2              # v row: [v[0:128] | 1 | v[128:256] | 1]
H = OUT_DIM // 2

F32 = mybir.dt.float32
BF16 = mybir.dt.bfloat16

N_WARM = 16                   # warm-up PSUM groups (2 junk matmuls each)


def _build():
    nc = bacc.Bacc(
        "TRN2",
        target_bir_lowering=False,
        debug=False,
        enable_asserts=False,
        num_devices=N_CORES,
    )
    # x pre-tiled on host: xh[b, h, p, i, s] = x[global_batch, h*512+s, i*128+p]
    # (p-major so each DMA partition row is I_T*512*2B = 4KB contiguous)
    xh = nc.dram_tensor(
        "xh", [B_LOC, SC, 128, I_T, 512], BF16, kind="ExternalInput"
    ).ap()
    # weights pre-tiled on host: w[p, i, o] = w_rot.T[i*128+p, o]
    # (2KB contiguous per partition row -> efficient DMA packets)
    wq = nc.dram_tensor("wq", [128, I_T, OUT_DIM], BF16, kind="ExternalInput").ap()
    wk = nc.dram_tensor("wk", [128, I_T, OUT_DIM], BF16, kind="ExternalInput").ap()
    wv = nc.dram_tensor("wv", [128, I_T, OUT_DIM], BF16, kind="ExternalInput").ap()
    bqr = nc.dram_tensor("bqr", [128, O_T], F32, kind="ExternalInput").ap()
    bkr = nc.dram_tensor("bkr", [128, O_T], F32, kind="ExternalInput").ap()
    bv = nc.dram_tensor("bv", [OUT_DIM], F32, kind="ExternalInput").ap()
    out = nc.dram_tensor("out", [B_LOC, S, OUT_DIM], F32, kind="ExternalOutput").ap()

    with tile.TileContext(nc) as tc, ExitStack() as ctx:
        sb = ctx.enter_context(tc.tile_pool(name="sb", bufs=1))
        small = ctx.enter_context(tc.tile_pool(name="small", bufs=4))
        ps_pj = ctx.enter_context(tc.tile_pool(name="ps_pj", bufs=3, space="PSUM"))
        ps_s = ctx.enter_context(tc.tile_pool(name="ps_s", bufs=3, space="PSUM"))
        ps_av = ctx.enter_context(tc.tile_pool(name="ps_av", bufs=2, space="PSUM"))

        S_, G_, C_ = nc.sync, nc.gpsimd, nc.scalar

        xsb = {}

        def load_xi(eng, b, h, i):
            t = sb.tile([128, 512], BF16, name=f"x{b}h{h}i{i}")
            eng.dma_start(out=t, in_=xh[b, h, :, i, :])
            xsb[b, h, i] = t

        def xt(b, h, i):
            return xsb[b, h, i]

        # --- DMA trigger schedule: first-needed-first, FIFO per queue,
        # byte-balanced, and large-row tiles: DMA throughput is packet-rate
        # bound (~90GB/s at 1KB rows, ~2x/4x better at 2KB/4KB rows), so x
        # loads use i-pair (2KB row) or full-h (4KB row) tiles. ---
        def load_xp(eng, b, h, half):
            """[128, 2, 512] i-pair chunk (2KB contiguous rows)."""
            t = sb.tile([128, 2, 512], BF16, name=f"x{b}h{h}p{half}")
            eng.dma_start(out=t, in_=xh[b, h, :, 2 * half:2 * half + 2, :])
            for i in (0, 1):
                xsb[b, h, 2 * half + i] = t[:, i, :]

        def load_xc(eng, b, h):
            """[128, I_T, 512] full-h tile (4KB contiguous rows)."""
            t = sb.tile([128, I_T, 512], BF16, name=f"x{b}h{h}")
            eng.dma_start(out=t, in_=xh[b, h, :, :, :])
            for i in range(I_T):
                xsb[b, h, i] = t[:, i, :]

        # sync  (HWDGE): wq wk x01          (1.02MB)
        # scalar(HWDGE): x00a x00b bqr bkr x11   (1.03MB)
        # gpsimd(SWDGE): wv bv x10          (0.90MB)
        wq_sb = sb.tile([128, I_T, OUT_DIM], BF16, name="wq_sb")
        S_.dma_start(out=wq_sb, in_=wq)
        load_xp(C_, 0, 0, 0)
        wv_sb = sb.tile([128, I_T, OUT_DIM], BF16, name="wv_sb")
        G_.dma_start(out=wv_sb, in_=wv)
        wk_sb = sb.tile([128, I_T, OUT_DIM], BF16, name="wk_sb")
        S_.dma_start(out=wk_sb, in_=wk)
        load_xp(C_, 0, 0, 1)
        bqr_sb = sb.tile([128, O_T], F32, name="bqr_sb")
        C_.dma_start(out=bqr_sb, in_=bqr)
        bkr_sb = sb.tile([128, O_T], F32, name="bkr_sb")
        C_.dma_start(out=bkr_sb, in_=bkr)
        bv_bc = sb.tile([128, OUT_DIM], F32, name="bv_bc")
        G_.dma_start(
            out=bv_bc,
            in_=bass.AP(tensor=bv.tensor, offset=bv.offset,
                        ap=[[0, 128], bv.ap[0]]))
        load_xc(S_, 0, 1)
        load_xc(C_, 1, 1)
        load_xc(G_, 1, 0)

        q_sb = [sb.tile([128, O_T, S], BF16, name=f"q{b}") for b in range(B_LOC)]
        k_sb = [sb.tile([128, O_T, S], BF16, name=f"k{b}") for b in range(B_LOC)]
        v_sb = [
            sb.tile([128, S_TILES, VW], BF16, name=f"v{b}")
            for b in range(B_LOC)
        ]
        e_sb = [sb.tile([128, S_TILES, S], BF16, name=f"e{b}") for b in range(B_LOC)]

        # ---- work-unit generators (each unit = one PSUM group + epilogue) ----
        def qk_unit(b, h, w_s, b_s, dst, o):
            ps = ps_pj.tile([128, 512], F32, tag="pj", name="pspj")
            for n, i in enumerate(range(I_T)):
                nc.tensor.matmul(
                    ps,
                    w_s[:, i, o * 128:(o + 1) * 128],
                    xt(b, h, i),
                    start=(n == 0),
                    stop=(n == I_T - 1),
                )
            nc.vector.tensor_scalar_add(
                dst[:, o, h * 512:(h + 1) * 512], ps, b_s[:, o:o + 1],
            )

        def v_unit(b, h, j):
            st = h * (S_TILES // SC) + j
            ps = ps_pj.tile([128, OUT_DIM], F32, tag="pj", name="pspj")
            for i in range(I_T):
                nc.tensor.matmul(
                    ps,
                    xt(b, h, i)[:, j * 128:(j + 1) * 128],
                    wv_sb[:, i, :],
                    start=(i == 0),
                    stop=(i == I_T - 1),
                )
            nc.vector.tensor_add(v_sb[b][:, st, 0:H], ps[:, 0:H],
                                 bv_bc[:, 0:H])
            nc.vector.tensor_add(v_sb[b][:, st, H + 1:VW - 1], ps[:, H:OUT_DIM],
                                 bv_bc[:, H:OUT_DIM])

        def qkv_units(b):
            units = []
            for h in range(SC):
                for w_s, b_s, dst in (
                    (wq_sb, bqr_sb, q_sb[b]),
                    (wk_sb, bkr_sb, k_sb[b]),
                ):
                    for o in range(O_T):
                        units.append(
                            lambda h=h, o=o, w_s=w_s, b_s=b_s, dst=dst:
                            qk_unit(b, h, w_s, b_s, dst, o))
                for j in range(S_TILES // SC):
                    units.append(lambda h=h, j=j: v_unit(b, h, j))
            return units

        def st_units(b):
            # scores^T tile (sk on partitions, sq chunk on free) + fused exp
            units = []
            for h in range(SC):
                for sk in range(S_TILES):
                    def f(h=h, sk=sk):
                        ps = ps_s.tile([128, 512], F32, tag="s", name="pss")
                        for d in range(O_T):
                            nc.tensor.matmul(
                                ps,
                                k_sb[b][:, d, sk * 128:(sk + 1) * 128],
                                q_sb[b][:, d, h * 512:(h + 1) * 512],
                                start=(d == 0),
                                stop=(d == O_T - 1),
                            )
                        nc.scalar.activation(
                            out=e_sb[b][:, sk, h * 512:(h + 1) * 512],
                            in_=ps,
                            func=mybir.ActivationFunctionType.Exp,
                            scale=SCALE,
                        )
                    units.append(f)
            return units

        OUT_Q = [S_, C_, G_]

        def av_units(b, split_last=False):
            units = []
            for sq in range(S_TILES):
                if split_last and sq == S_TILES - 1:
                    def f(sq=sq):
                        # column-split final unit: half A finishes first and
                        # its divide+store overlaps half B's matmuls.
                        psA = ps_av.tile([128, H + 1], F32, tag="av", name="psav")
                        for sk in range(S_TILES):
                            nc.tensor.matmul(
                                psA,
                                e_sb[b][:, sk, sq * 128:(sq + 1) * 128],
                                v_sb[b][:, sk, 0:H + 1],
                                start=(sk == 0),
                                stop=(sk == S_TILES - 1),
                            )
                        psB = ps_av.tile([128, H + 1], F32, tag="av", name="psav")
                        for sk in range(S_TILES):
                            nc.tensor.matmul(
                                psB,
                                e_sb[b][:, sk, sq * 128:(sq + 1) * 128],
                                v_sb[b][:, sk, H + 1:VW],
                                start=(sk == 0),
                                stop=(sk == S_TILES - 1),
                            )
                        recA = small.tile([128, 1], F32, tag="rec", name="rec")
                        nc.vector.reciprocal(recA, psA[:, H:H + 1])
                        otA = small.tile([128, H], F32, tag="ot", name="ot")
                        nc.vector.tensor_scalar_mul(otA, psA[:, 0:H], recA)
                        nc.sync.dma_start(
                            out=out[b, sq * 128:(sq + 1) * 128, 0:H], in_=otA)
                        recB = small.tile([128, 1], F32, tag="rec", name="rec")
                        nc.vector.reciprocal(recB, psB[:, H:H + 1])
                        otB = small.tile([128, H], F32, tag="ot", name="ot")
                        nc.vector.tensor_scalar_mul(otB[:, 0:H // 2],
                                                    psB[:, 0:H // 2], recB)
                        nc.scalar.dma_start(
                            out=out[b, sq * 128:(sq + 1) * 128, H:H + H // 2],
                            in_=otB[:, 0:H // 2])
                        nc.vector.tensor_scalar_mul(otB[:, H // 2:H],
                                                    psB[:, H // 2:H], recB)
                        nc.sync.dma_start(
                            out=out[b, sq * 128:(sq + 1) * 128, H + H // 2:OUT_DIM],
                            in_=otB[:, H // 2:H])
                    units.append(f)
                    continue

                def f(sq=sq):
                    ps = ps_av.tile([128, VW], F32, tag="av", name="psav")
                    for sk in range(S_TILES):
                        nc.tensor.matmul(
                            ps,
                            e_sb[b][:, sk, sq * 128:(sq + 1) * 128],
                            v_sb[b][:, sk, :],
                            start=(sk == 0),
                            stop=(sk == S_TILES - 1),
                        )
                    rec = small.tile([128, 1], F32, tag="rec", name="rec")
                    nc.vector.reciprocal(rec, ps[:, H:H + 1])
                    ot = small.tile([128, OUT_DIM], F32, tag="ot", name="ot")
                    nc.vector.tensor_scalar_mul(ot[:, 0:H], ps[:, 0:H], rec)
                    nc.vector.tensor_scalar_mul(ot[:, H:OUT_DIM],
                                                ps[:, H + 1:VW - 1], rec)
                    OUT_Q[(b * S_TILES + sq) % 3].dma_start(
                        out=out[b, sq * 128:(sq + 1) * 128, :], in_=ot
                    )
                units.append(f)
            return units

        # ---- PE warm-up: keep TensorE busy while input DMAs land so the
        # HAM clock ramp starts early and never resets; sized to hand off
        # to the first real matmul with no gap ----
        junk = sb.tile([128, 128], BF16, name="junk")
        nc.vector.memset(junk, 0.0)
        for _ in range(N_WARM):
            wps = ps_av.tile([128, 128], F32, tag="av", name="warmps")
            for _k in range(2):
                nc.tensor.matmul(wps, junk, junk, start=(_k == 0),
                                 stop=(_k == 1))

        # ones-planes of v (cols H and VW-1), never overwritten by v units
        for b in range(B_LOC):
            nc.vector.memset(v_sb[b][:, :, H:H + 1], 1.0)
            nc.vector.memset(v_sb[b][:, :, VW - 1:VW], 1.0)

        # ---- emission: st0 (no DMA deps) runs before qkv1 so the b=1
        # x tiles get ~7us more delivery slack; PE executes in-order, so a
        # single late DMA dep would stall it and down-throttle the clock ----
        for u in qkv_units(0):
            u()
        for a in st_units(0):
            a()
        for u in qkv_units(1):
            u()
        st1 = st_units(1)
        av0 = av_units(0)
        for idx in range(S_TILES):
            st1[2 * idx]()
            st1[2 * idx + 1]()
            av0[idx]()
        for u in av_units(1, split_last=True):
            u()

    nc.compile()
    return nc


_CACHE = {}


def _get_nc():
    if "nc" not in _CACHE:
        _CACHE["nc"] = _build()
    return _CACHE["nc"]


def _rope_fold(w, bvec):
    """Fold the (position-independent) RoPE rotation into weights/bias.

    Mirrors the reference: inv_freq over arange(0, OUT_DIM, 2)/OUT_DIM,
    angle = 2*S*inv_freq, pairs (2j, 2j+1) rotated by angle_j.
    Computed in float32 to track the reference's f32 arithmetic.
    """
    exps = np.arange(0, OUT_DIM, 2, dtype=np.float32) / np.float32(OUT_DIM)
    inv = (np.float32(1.0) / np.power(np.float32(THETA), exps)).astype(np.float32)
    ang = (np.float32(2.0 * S) * inv).astype(np.float32)
    cos = np.cos(ang).astype(np.float32)
    sin = np.sin(ang).astype(np.float32)

    w2 = w.reshape(OUT_DIM // 2, 2, IN_DIM)
    wr = np.empty_like(w2)
    wr[:, 0] = cos[:, None] * w2[:, 0] - sin[:, None] * w2[:, 1]
    wr[:, 1] = sin[:, None] * w2[:, 0] + cos[:, None] * w2[:, 1]
    b2 = bvec.reshape(OUT_DIM // 2, 2)
    br = np.empty_like(b2)
    br[:, 0] = cos * b2[:, 0] - sin * b2[:, 1]
    br[:, 1] = sin * b2[:, 0] + cos * b2[:, 1]
    return wr.reshape(OUT_DIM, IN_DIM), br.reshape(OUT_DIM)


def _pack_w(w_rot, bf16):
    """[OUT_DIM, IN_DIM] weight -> [128, I_T, OUT_DIM] bf16 SBUF tile layout."""
    wt = np.ascontiguousarray(w_rot.T)                  # [IN_DIM, OUT_DIM]
    return np.ascontiguousarray(
        wt.reshape(I_T, 128, OUT_DIM).transpose(1, 0, 2)
    ).astype(bf16)


def _prep_inputs(x, wq, bq, wk, bk, wv, bv):
    import ml_dtypes
    bf16 = ml_dtypes.bfloat16
    x = np.asarray(x, dtype=np.float32)
    wq_r, bq_r = _rope_fold(np.asarray(wq, np.float32), np.asarray(bq, np.float32))
    wk_r, bk_r = _rope_fold(np.asarray(wk, np.float32), np.asarray(bk, np.float32))
    wv = np.asarray(wv, np.float32)
    bv = np.asarray(bv, np.float32)

    shared = {
        "wq": _pack_w(wq_r, bf16),
        "wk": _pack_w(wk_r, bf16),
        "wv": _pack_w(wv, bf16),
        "bqr": np.ascontiguousarray(bq_r.reshape(O_T, 128).T),
        "bkr": np.ascontiguousarray(bk_r.reshape(O_T, 128).T),
        "bv": bv,
    }
    in_maps = []
    for c in range(N_CORES):
        shard = x[c * B_LOC:(c + 1) * B_LOC]            # [B_LOC, S, IN_DIM]
        # xh[b, h, p, i, s] = shard[b, h*512+s, i*128+p]
        xh = np.ascontiguousarray(
            shard.transpose(0, 2, 1)                     # [b, IN_DIM, S]
            .reshape(B_LOC, I_T, 128, SC, 512)
            .transpose(0, 3, 2, 1, 4)
        ).astype(bf16)
        in_maps.append({"xh": xh, **shared})
    return in_maps


def _execute(in_maps, trace=False, tmpdir=None):
    nc = _get_nc()
    return run_bass_kernel_spmd(
        nc, in_maps, core_ids=list(range(N_CORES)), trace=trace, tmpdir=tmpdir
    )


def kernel(x, wq, bq, wk, bk, wv, bv):
    in_maps = _prep_inputs(x, wq, bq, wk, bk, wv, bv)
    res = _execute(in_maps)
    return np.concatenate(
        [np.asarray(res.results[i]["out"]) for i in range(N_CORES)], axis=0
    ).astype(np.float32)


# revision 24
# speedup vs baseline: 1.1764x; 1.1764x over previous
"""Trainium2 Bass kernel: batch-sharded fused attention (nn_Attention_48893907698232).

Reference computation (per batch b):
    q = x @ wq.T + bq ; k = x @ wk.T + bk ; v = x @ wv.T + bv
    q, k <- fixed RoPE rotation (same rotation at every position)
    out  = softmax((q @ k.T) / sqrt(512)) @ v

Strategy (v1 -> v2 changes marked *):
  * Data-parallel over batch: 16 batches -> 2 per NeuronCore, no collectives.
  * RoPE folded into projection weights/biases on the host.
  * Host pre-transposes/pre-tiles x and weights so every matmul contraction
    dim lands on SBUF partitions with zero on-chip transposes:
      qT[o,s]  = sum_i wqT[i,o] * xT[i,s]     (PSUM: o on partitions)
      kT[o,s]  likewise
      v[s,o]   = sum_i xT[i,s] * wvT[i,o]     (PSUM: s on partitions)
      ST[sk,sq]= sum_d kT[d,sk] * qT[d,sq]    -> exp(scale*ST) on ACT -> ET
      out[sq,:]= sum_sk ET[sk,sq] * [vA|1|vB|1]  (ones columns => softmax
                                                  rowsum in cols 128/257)
      out  <- out[:, 0:128 & 129:257] * (1 / rowsum)
  * Softmax runs without max-subtraction (scores ~N(0,0.75), exp safe).
  * Matmul operands bf16 (fp32 PSUM accumulate).
* Startup: wq/wk split into 128-col halves so the first matmul's weight
    dependency is 128KB, first-needed tiles are FIFO-ordered across the
    three DMA trigger queues (sync/scalar HWDGE + gpsimd SWDGE), and the
    first two q-units' contractions are interleaved to match DMA arrival.
* PE warm-up junk matmuls sized to hand off to real work with no gap
    (a PE-activity gap appears to reset the HAM clock-ramp timer).
* v is stored as [vA|1|vB|1] so the final AV unit can be column-split
    into two half-width PSUM accumulations: the first half's epilogue +
    output DMA overlap the second half's matmuls, shrinking the tail.
"""

import math
import os
import sys

import numpy as np

os.environ.setdefault("MYCRO_LOCAL_CACHE", "1")
if "/opt/trn_rl_repo" not in sys.path:
    sys.path.insert(0, "/opt/trn_rl_repo")

from contextlib import ExitStack

import concourse.bass as bass
import concourse.tile as tile
from concourse import bacc, mybir
from concourse.bass_utils import run_bass_kernel_spmd

B, S, IN_DIM, OUT_DIM = 16, 1024, 512, 256
THETA = 10000.0
N_CORES = 8
B_LOC = B // N_CORES          # batches per core
I_T = IN_DIM // 128           # 4 contraction tiles for the projections
O_T = OUT_DIM // 128          # 2 feature tiles
S_TILES = S // 128            # 8 sequence tiles
SC = S // 512                 # 2 sequence chunks of 512
SCALE = 1.0 / math.sqrt(IN_DIM)
VW = OUT_DIM + 2              # v row: [v[0:128] | 1 | v[128:256] | 1]
H = OUT_DIM // 2

F32 = mybir.dt.float32
BF16 = mybir.dt.bfloat16

N_WARM = 19                   # warm-up PSUM groups (2 junk matmuls each)


def _build():
    nc = bacc.Bacc(
        "TRN2",
        target_bir_lowering=False,
        debug=False,
        enable_asserts=False,
        num_devices=N_CORES,
    )
    # x pre-tiled on host: xh[b, h, p, i, s] = x[global_batch, h*512+s, i*128+p]
    # (p-major so each DMA partition row is I_T*512*2B = 4KB contiguous)
    xh = nc.dram_tensor(
        "xh", [B_LOC, SC, 128, I_T, 512], BF16, kind="ExternalInput"
    ).ap()
    # weights pre-tiled on host: w[p, i, o] = w_rot.T[i*128+p, o]
    # (2KB contiguous per partition row -> efficient DMA packets)
    wq = nc.dram_tensor("wq", [128, I_T, OUT_DIM], BF16, kind="ExternalInput").ap()
    wk = nc.dram_tensor("wk", [128, I_T, OUT_DIM], BF16, kind="ExternalInput").ap()
    wv = nc.dram_tensor("wv", [128, I_T, OUT_DIM], BF16, kind="ExternalInput").ap()
    # all biases in one tensor with 1040B rows: tiny-row DMAs (8B rows) cost
    # ~10ns/packet fixed and poison a queue for ~3us; one fat DMA is ~0.5us.
    # bias[p] = [bqr[p, 0:2] | bkr[p, 0:2] | bv_broadcast[0:256]]
    bias = nc.dram_tensor("bias", [128, 4 + OUT_DIM], F32,
                          kind="ExternalInput").ap()
    out = nc.dram_tensor("out", [B_LOC, S, OUT_DIM], F32, kind="ExternalOutput").ap()

    with tile.TileContext(nc) as tc, ExitStack() as ctx:
        sb = ctx.enter_context(tc.tile_pool(name="sb", bufs=1))
        small = ctx.enter_context(tc.tile_pool(name="small", bufs=4))
        ps_pj = ctx.enter_context(tc.tile_pool(name="ps_pj", bufs=3, space="PSUM"))
        ps_s = ctx.enter_context(tc.tile_pool(name="ps_s", bufs=3, space="PSUM"))
        ps_av = ctx.enter_context(tc.tile_pool(name="ps_av", bufs=2, space="PSUM"))

        S_, G_, C_ = nc.sync, nc.gpsimd, nc.scalar

        xsb = {}

        def load_xi(eng, b, h, i):
            t = sb.tile([128, 512], BF16, name=f"x{b}h{h}i{i}")
            eng.dma_start(out=t, in_=xh[b, h, :, i, :])
            xsb[b, h, i] = t

        def xt(b, h, i):
            return xsb[b, h, i]

        # --- DMA trigger schedule: first-needed-first, FIFO per queue,
        # byte-balanced, and large-row tiles: DMA throughput is packet-rate
        # bound (~90GB/s at 1KB rows, ~2x/4x better at 2KB/4KB rows), so x
        # loads use i-pair (2KB row) or full-h (4KB row) tiles. ---
        def load_xp(eng, b, h, half):
            """[128, 2, 512] i-pair chunk (2KB contiguous rows)."""
            t = sb.tile([128, 2, 512], BF16, name=f"x{b}h{h}p{half}")
            eng.dma_start(out=t, in_=xh[b, h, :, 2 * half:2 * half + 2, :])
            for i in (0, 1):
                xsb[b, h, 2 * half + i] = t[:, i, :]

        def load_xc(eng, b, h):
            """[128, I_T, 512] full-h tile (4KB contiguous rows)."""
            t = sb.tile([128, I_T, 512], BF16, name=f"x{b}h{h}")
            eng.dma_start(out=t, in_=xh[b, h, :, :, :])
            for i in range(I_T):
                xsb[b, h, i] = t[:, i, :]

        # sync  (HWDGE): wqA x00i1 wqB wk      (0.64MB, then free for outputs)
        # scalar(HWDGE): x00i0 x00b x01 x11    (1.40MB)
        # gpsimd(SWDGE): wv bias x10           (0.90MB)
        # wq split into i-pair halves so the paired first q-unit can start
        # on wqA + x00i0 alone (~11.5us) and consume tiles as they land.
        wq_sb = sb.tile([128, I_T, OUT_DIM], BF16, name="wq_sb")
        S_.dma_start(out=wq_sb[:, 0:2, :], in_=wq[:, 0:2, :])
        load_xi(C_, 0, 0, 0)
        wv_sb = sb.tile([128, I_T, OUT_DIM], BF16, name="wv_sb")
        G_.dma_start(out=wv_sb, in_=wv)
        load_xi(S_, 0, 0, 1)
        S_.dma_start(out=wq_sb[:, 2:4, :], in_=wq[:, 2:4, :])
        load_xp(C_, 0, 0, 1)
        bias_sb = sb.tile([128, 4 + OUT_DIM], F32, name="bias_sb")
        G_.dma_start(out=bias_sb, in_=bias)
        bqr_sb = bias_sb[:, 0:O_T]
        bkr_sb = bias_sb[:, O_T:2 * O_T]
        bv_bc = bias_sb[:, 4:4 + OUT_DIM]
        wk_sb = sb.tile([128, I_T, OUT_DIM], BF16, name="wk_sb")
        S_.dma_start(out=wk_sb, in_=wk)
        load_xc(C_, 0, 1)
        load_xc(G_, 1, 0)
        load_xc(C_, 1, 1)

        q_sb = [sb.tile([128, O_T, S], BF16, name=f"q{b}") for b in range(B_LOC)]
        k_sb = [sb.tile([128, O_T, S], BF16, name=f"k{b}") for b in range(B_LOC)]
        v_sb = [
            sb.tile([128, S_TILES, VW], BF16, name=f"v{b}")
            for b in range(B_LOC)
        ]
        e_sb = [sb.tile([128, S_TILES, S], BF16, name=f"e{b}") for b in range(B_LOC)]

        # ---- work-unit generators (each unit = one PSUM group + epilogue) ----
        def qk_unit(b, h, w_s, b_s, dst, o):
            ps = ps_pj.tile([128, 512], F32, tag="pj", name="pspj")
            for n, i in enumerate(range(I_T)):
                nc.tensor.matmul(
                    ps,
                    w_s[:, i, o * 128:(o + 1) * 128],
                    xt(b, h, i),
                    start=(n == 0),
                    stop=(n == I_T - 1),
                )
            nc.vector.tensor_scalar_add(
                dst[:, o, h * 512:(h + 1) * 512], ps, b_s[:, o:o + 1],
            )

        def v_unit(b, h, j):
            st = h * (S_TILES // SC) + j
            ps = ps_pj.tile([128, OUT_DIM], F32, tag="pj", name="pspj")
            for i in range(I_T):
                nc.tensor.matmul(
                    ps,
                    xt(b, h, i)[:, j * 128:(j + 1) * 128],
                    wv_sb[:, i, :],
                    start=(i == 0),
                    stop=(i == I_T - 1),
                )
            nc.vector.tensor_add(v_sb[b][:, st, 0:H], ps[:, 0:H],
                                 bv_bc[:, 0:H])
            nc.vector.tensor_add(v_sb[b][:, st, H + 1:VW - 1], ps[:, H:OUT_DIM],
                                 bv_bc[:, H:OUT_DIM])

        def q00_pair():
            # first two q-units with interleaved contractions: one x tile is
            # consumed per ~1.2us (half clock), matching DMA delivery, so the
            # PE never gaps while the rest of the inputs stream in.
            psA = ps_pj.tile([128, 512], F32, tag="pj", name="pspj")
            psB = ps_pj.tile([128, 512], F32, tag="pj", name="pspj")
            for n in range(I_T):
                nc.tensor.matmul(psA, wq_sb[:, n, 0:128], xt(0, 0, n),
                                 start=(n == 0), stop=(n == I_T - 1))
                nc.tensor.matmul(psB, wq_sb[:, n, 128:256], xt(0, 0, n),
                                 start=(n == 0), stop=(n == I_T - 1))
            nc.vector.tensor_scalar_add(q_sb[0][:, 0, 0:512], psA,
                                        bqr_sb[:, 0:1])
            nc.vector.tensor_scalar_add(q_sb[0][:, 1, 0:512], psB,
                                        bqr_sb[:, 1:2])

        def qkv_units(b):
            units = []
            for h in range(SC):
                for w_s, b_s, dst in (
                    (wq_sb, bqr_sb, q_sb[b]),
                    (wk_sb, bkr_sb, k_sb[b]),
                ):
                    if b == 0 and h == 0 and w_s is wq_sb:
                        units.append(q00_pair)
                        continue
                    for o in range(O_T):
                        units.append(
                            lambda h=h, o=o, w_s=w_s, b_s=b_s, dst=dst:
                            qk_unit(b, h, w_s, b_s, dst, o))
                for j in range(S_TILES // SC):
                    units.append(lambda h=h, j=j: v_unit(b, h, j))
            return units

        def st_units(b):
            # scores^T tile (sk on partitions, sq chunk on free) + fused exp
            units = []
            for h in range(SC):
                for sk in range(S_TILES):
                    def f(h=h, sk=sk):
                        ps = ps_s.tile([128, 512], F32, tag="s", name="pss")
                        for d in range(O_T):
                            nc.tensor.matmul(
                                ps,
                                k_sb[b][:, d, sk * 128:(sk + 1) * 128],
                                q_sb[b][:, d, h * 512:(h + 1) * 512],
                                start=(d == 0),
                                stop=(d == O_T - 1),
                            )
                        nc.scalar.activation(
                            out=e_sb[b][:, sk, h * 512:(h + 1) * 512],
                            in_=ps,
                            func=mybir.ActivationFunctionType.Exp,
                            scale=SCALE,
                        )
                    units.append(f)
            return units

        OUT_Q = [S_, C_]      # HWDGE only; SWDGE prep/trigger is slower

        def av_units(b, split_last=False):
            units = []
            for sq in range(S_TILES):
                if split_last and sq == S_TILES - 1:
                    def f(sq=sq):
                        # column-split final unit: half A finishes first and
                        # its divide+store overlaps half B's matmuls.
                        psA = ps_av.tile([128, H + 1], F32, tag="av", name="psav")
                        for sk in range(S_TILES):
                            nc.tensor.matmul(
                                psA,
                                e_sb[b][:, sk, sq * 128:(sq + 1) * 128],
                                v_sb[b][:, sk, 0:H + 1],
                                start=(sk == 0),
                                stop=(sk == S_TILES - 1),
                            )
                        psB = ps_av.tile([128, H + 1], F32, tag="av", name="psav")
                        for sk in range(S_TILES):
                            nc.tensor.matmul(
                                psB,
                                e_sb[b][:, sk, sq * 128:(sq + 1) * 128],
                                v_sb[b][:, sk, H + 1:VW],
                                start=(sk == 0),
                                stop=(sk == S_TILES - 1),
                            )
                        recA = small.tile([128, 1], F32, tag="rec", name="rec")
                        nc.vector.reciprocal(recA, psA[:, H:H + 1])
                        otA = small.tile([128, H], F32, tag="ot", name="ot")
                        nc.vector.tensor_scalar_mul(otA, psA[:, 0:H], recA)
                        nc.sync.dma_start(
                            out=out[b, sq * 128:(sq + 1) * 128, 0:H], in_=otA)
                        recB = small.tile([128, 1], F32, tag="rec", name="rec")
                        nc.vector.reciprocal(recB, psB[:, H:H + 1])
                        otB = small.tile([128, H], F32, tag="ot", name="ot")
                        nc.vector.tensor_scalar_mul(otB[:, 0:H // 2],
                                                    psB[:, 0:H // 2], recB)
                        nc.scalar.dma_start(
                            out=out[b, sq * 128:(sq + 1) * 128, H:H + H // 2],
                            in_=otB[:, 0:H // 2])
                        nc.vector.tensor_scalar_mul(otB[:, H // 2:H],
                                                    psB[:, H // 2:H], recB)
                        nc.sync.dma_start(
                            out=out[b, sq * 128:(sq + 1) * 128, H + H // 2:OUT_DIM],
                            in_=otB[:, H // 2:H])
                    units.append(f)
                    continue

                def f(sq=sq):
                    ps = ps_av.tile([128, VW], F32, tag="av", name="psav")
                    for sk in range(S_TILES):
                        nc.tensor.matmul(
                            ps,
                            e_sb[b][:, sk, sq * 128:(sq + 1) * 128],
                            v_sb[b][:, sk, :],
                            start=(sk == 0),
                            stop=(sk == S_TILES - 1),
                        )
                    rec = small.tile([128, 1], F32, tag="rec", name="rec")
                    nc.vector.reciprocal(rec, ps[:, H:H + 1])
                    ot = small.tile([128, OUT_DIM], F32, tag="ot", name="ot")
                    nc.vector.tensor_scalar_mul(ot[:, 0:H], ps[:, 0:H], rec)
                    nc.vector.tensor_scalar_mul(ot[:, H:OUT_DIM],
                                                ps[:, H + 1:VW - 1], rec)
                    OUT_Q[(b * S_TILES + sq) % 2].dma_start(
                        out=out[b, sq * 128:(sq + 1) * 128, :], in_=ot
                    )
                units.append(f)
            return units

        # ---- PE warm-up: keep TensorE busy while input DMAs land so the
        # HAM clock ramp starts early and never resets; sized to hand off
        # to the first real matmul with no gap ----
        junk = sb.tile([128, 128], BF16, name="junk")
        nc.vector.memset(junk, 0.0)
        for _ in range(N_WARM):
            wps = ps_av.tile([128, 128], F32, tag="av", name="warmps")
            for _k in range(2):
                nc.tensor.matmul(wps, junk, junk, start=(_k == 0),
                                 stop=(_k == 1))

        # ones-planes of v (cols H and VW-1), never overwritten by v units
        for b in range(B_LOC):
            nc.vector.memset(v_sb[b][:, :, H:H + 1], 1.0)
            nc.vector.memset(v_sb[b][:, :, VW - 1:VW], 1.0)

        # ---- emission: st0 (no DMA deps) runs before qkv1 so the b=1
        # x tiles get ~7us more delivery slack; PE executes in-order, so a
        # single late DMA dep would stall it and down-throttle the clock ----
        for u in qkv_units(0):
            u()
        for a in st_units(0):
            a()
        for u in qkv_units(1):
            u()
        st1 = st_units(1)
        av0 = av_units(0)
        for idx in range(S_TILES):
            st1[2 * idx]()
            st1[2 * idx + 1]()
            av0[idx]()
        for u in av_units(1, split_last=True):
            u()

    nc.compile()
    return nc


_CACHE = {}


def _get_nc():
    if "nc" not in _CACHE:
        _CACHE["nc"] = _build()
    return _CACHE["nc"]


def _rope_fold(w, bvec):
    """Fold the (position-independent) RoPE rotation into weights/bias.

    Mirrors the reference: inv_freq over arange(0, OUT_DIM, 2)/OUT_DIM,
    angle = 2*S*inv_freq, pairs (2j, 2j+1) rotated by angle_j.
    Computed in float32 to track the reference's f32 arithmetic.
    """
    exps = np.arange(0, OUT_DIM, 2, dtype=np.float32) / np.float32(OUT_DIM)
    inv = (np.float32(1.0) / np.power(np.float32(THETA), exps)).astype(np.float32)
    ang = (np.float32(2.0 * S) * inv).astype(np.float32)
    cos = np.cos(ang).astype(np.float32)
    sin = np.sin(ang).astype(np.float32)

    w2 = w.reshape(OUT_DIM // 2, 2, IN_DIM)
    wr = np.empty_like(w2)
    wr[:, 0] = cos[:, None] * w2[:, 0] - sin[:, None] * w2[:, 1]
    wr[:, 1] = sin[:, None] * w2[:, 0] + cos[:, None] * w2[:, 1]
    b2 = bvec.reshape(OUT_DIM // 2, 2)
    br = np.empty_like(b2)
    br[:, 0] = cos * b2[:, 0] - sin * b2[:, 1]
    br[:, 1] = sin * b2[:, 0] + cos * b2[:, 1]
    return wr.reshape(OUT_DIM, IN_DIM), br.reshape(OUT_DIM)


def _pack_w(w_rot, bf16):
    """[OUT_DIM, IN_DIM] weight -> [128, I_T, OUT_DIM] bf16 SBUF tile layout."""
    wt = np.ascontiguousarray(w_rot.T)                  # [IN_DIM, OUT_DIM]
    return np.ascontiguousarray(
        wt.reshape(I_T, 128, OUT_DIM).transpose(1, 0, 2)
    ).astype(bf16)


def _prep_inputs(x, wq, bq, wk, bk, wv, bv):
    import ml_dtypes
    bf16 = ml_dtypes.bfloat16
    x = np.asarray(x, dtype=np.float32)
    wq_r, bq_r = _rope_fold(np.asarray(wq, np.float32), np.asarray(bq, np.float32))
    wk_r, bk_r = _rope_fold(np.asarray(wk, np.float32), np.asarray(bk, np.float32))
    wv = np.asarray(wv, np.float32)
    bv = np.asarray(bv, np.float32)

    bias = np.empty((128, 4 + OUT_DIM), np.float32)
    bias[:, 0:O_T] = bq_r.reshape(O_T, 128).T
    bias[:, O_T:2 * O_T] = bk_r.reshape(O_T, 128).T
    bias[:, 4:] = bv[None, :]
    shared = {
        "wq": _pack_w(wq_r, bf16),
        "wk": _pack_w(wk_r, bf16),
        "wv": _pack_w(wv, bf16),
        "bias": np.ascontiguousarray(bias),
    }
    in_maps = []
    for c in range(N_CORES):
        shard = x[c * B_LOC:(c + 1) * B_LOC]            # [B_LOC, S, IN_DIM]
        # xh[b, h, p, i, s] = shard[b, h*512+s, i*128+p]
        xh = np.ascontiguousarray(
            shard.transpose(0, 2, 1)                     # [b, IN_DIM, S]
            .reshape(B_LOC, I_T, 128, SC, 512)
            .transpose(0, 3, 2, 1, 4)
        ).astype(bf16)
        in_maps.append({"xh": xh, **shared})
    return in_maps


def _execute(in_maps, trace=False, tmpdir=None):
    nc = _get_nc()
    return run_bass_kernel_spmd(
        nc, in_maps, core_ids=list(range(N_CORES)), trace=trace, tmpdir=tmpdir
    )


def kernel(x, wq, bq, wk, bk, wv, bv):
    in_maps = _prep_inputs(x, wq, bq, wk, bk, wv, bv)
    res = _execute(in_maps)
    return np.concatenate(
        [np.asarray(res.results[i]["out"]) for i in range(N_CORES)], axis=0
    ).astype(np.float32)


# revision 30
# speedup vs baseline: 1.2509x; 1.0633x over previous
"""Trainium2 Bass kernel: batch-sharded fused attention (nn_Attention_48893907698232).

Reference computation (per batch b):
    q = x @ wq.T + bq ; k = x @ wk.T + bk ; v = x @ wv.T + bv
    q, k <- fixed RoPE rotation (same rotation at every position)
    out  = softmax((q @ k.T) / sqrt(512)) @ v

Strategy (v1 -> v2 changes marked *):
  * Data-parallel over batch: 16 batches -> 2 per NeuronCore, no collectives.
  * RoPE folded into projection weights/biases on the host.
  * Host pre-transposes/pre-tiles x and weights so every matmul contraction
    dim lands on SBUF partitions with zero on-chip transposes:
      qT[o,s]  = sum_i wqT[i,o] * xT[i,s]     (PSUM: o on partitions)
      kT[o,s]  likewise
      v[s,o]   = sum_i xT[i,s] * wvT[i,o]     (PSUM: s on partitions)
      ST[sk,sq]= sum_d kT[d,sk] * qT[d,sq]    -> exp(scale*ST) on ACT -> ET
      out[sq,:]= sum_sk ET[sk,sq] * [vA|1|vB|1]  (ones columns => softmax
                                                  rowsum in cols 128/257)
      out  <- out[:, 0:128 & 129:257] * (1 / rowsum)
  * Softmax runs without max-subtraction (scores ~N(0,0.75), exp safe).
  * Matmul operands bf16 (fp32 PSUM accumulate).
* Startup: wq/wk split into 128-col halves so the first matmul's weight
    dependency is 128KB, first-needed tiles are FIFO-ordered across the
    three DMA trigger queues (sync/scalar HWDGE + gpsimd SWDGE), and the
    first two q-units' contractions are interleaved to match DMA arrival.
* PE warm-up junk matmuls sized to hand off to real work with no gap
    (a PE-activity gap appears to reset the HAM clock-ramp timer).
* v is stored as [vA|1|vB|1] so the final AV unit can be column-split
    into two half-width PSUM accumulations: the first half's epilogue +
    output DMA overlap the second half's matmuls, shrinking the tail.
"""

import math
import os
import sys

import numpy as np

os.environ.setdefault("MYCRO_LOCAL_CACHE", "1")
if "/opt/trn_rl_repo" not in sys.path:
    sys.path.insert(0, "/opt/trn_rl_repo")

from contextlib import ExitStack

import concourse.bass as bass
import concourse.tile as tile
from concourse import bacc, mybir
from concourse.bass_utils import run_bass_kernel_spmd

B, S, IN_DIM, OUT_DIM = 16, 1024, 512, 256
THETA = 10000.0
N_CORES = 8
B_LOC = B // N_CORES          # batches per core
I_T = IN_DIM // 128           # 4 contraction tiles for the projections
O_T = OUT_DIM // 128          # 2 feature tiles
S_TILES = S // 128            # 8 sequence tiles
SC = S // 512                 # 2 sequence chunks of 512
SCALE = 1.0 / math.sqrt(IN_DIM)
VW = OUT_DIM + 2              # v row: [v[0:128] | 1 | v[128:256] | 1]
H = OUT_DIM // 2

F32 = mybir.dt.float32
BF16 = mybir.dt.bfloat16

N_WARM = 19                   # warm-up PSUM groups (2 junk matmuls each)


def _build():
    nc = bacc.Bacc(
        "TRN2",
        target_bir_lowering=False,
        debug=False,
        enable_asserts=False,
        num_devices=N_CORES,
    )
    # x pre-tiled on host: xh[b, h, p, i, s] = x[global_batch, h*512+s, i*128+p]
    # (p-major so each DMA partition row is I_T*512*2B = 4KB contiguous)
    xh = nc.dram_tensor(
        "xh", [B_LOC, SC, 128, I_T, 512], BF16, kind="ExternalInput"
    ).ap()
    # weights pre-tiled on host: w[p, i, o] = w_rot.T[i*128+p, o]
    # (2KB contiguous per partition row -> efficient DMA packets)
    wq = nc.dram_tensor("wq", [128, I_T, OUT_DIM], BF16, kind="ExternalInput").ap()
    wk = nc.dram_tensor("wk", [128, I_T, OUT_DIM], BF16, kind="ExternalInput").ap()
    wv = nc.dram_tensor("wv", [128, I_T, OUT_DIM], BF16, kind="ExternalInput").ap()
    # all biases in one tensor with 1040B rows: tiny-row DMAs (8B rows) cost
    # ~10ns/packet fixed and poison a queue for ~3us; one fat DMA is ~0.5us.
    # bias[p] = [bqr[p, 0:2] | bkr[p, 0:2] | bv_broadcast[0:256]]
    bias = nc.dram_tensor("bias", [128, 4 + OUT_DIM], F32,
                          kind="ExternalInput").ap()
    out = nc.dram_tensor("out", [B_LOC, S, OUT_DIM], F32, kind="ExternalOutput").ap()

    with tile.TileContext(nc) as tc, ExitStack() as ctx:
        sb = ctx.enter_context(tc.tile_pool(name="sb", bufs=1))
        small = ctx.enter_context(tc.tile_pool(name="small", bufs=8))
        ps_pj = ctx.enter_context(tc.tile_pool(name="ps_pj", bufs=3, space="PSUM"))
        ps_s = ctx.enter_context(tc.tile_pool(name="ps_s", bufs=3, space="PSUM"))
        ps_av = ctx.enter_context(tc.tile_pool(name="ps_av", bufs=2, space="PSUM"))

        S_, G_, C_ = nc.sync, nc.gpsimd, nc.scalar

        xsb = {}

        def load_xi(eng, b, h, i):
            t = sb.tile([128, 512], BF16, name=f"x{b}h{h}i{i}")
            eng.dma_start(out=t, in_=xh[b, h, :, i, :])
            xsb[b, h, i] = t

        def xt(b, h, i):
            return xsb[b, h, i]

        # --- DMA trigger schedule: first-needed-first, FIFO per queue,
        # byte-balanced, and large-row tiles: DMA throughput is packet-rate
        # bound (~90GB/s at 1KB rows, ~2x/4x better at 2KB/4KB rows), so x
        # loads use i-pair (2KB row) or full-h (4KB row) tiles. ---
        def load_xp(eng, b, h, half):
            """[128, 2, 512] i-pair chunk (2KB contiguous rows)."""
            t = sb.tile([128, 2, 512], BF16, name=f"x{b}h{h}p{half}")
            eng.dma_start(out=t, in_=xh[b, h, :, 2 * half:2 * half + 2, :])
            for i in (0, 1):
                xsb[b, h, 2 * half + i] = t[:, i, :]

        def load_xc(eng, b, h):
            """[128, I_T, 512] full-h tile (4KB contiguous rows)."""
            t = sb.tile([128, I_T, 512], BF16, name=f"x{b}h{h}")
            eng.dma_start(out=t, in_=xh[b, h, :, :, :])
            for i in range(I_T):
                xsb[b, h, i] = t[:, i, :]

        # sync  (HWDGE): wq x00b x11           (1.02MB)
        # scalar(HWDGE): x00a wk x01           (1.02MB)
        # gpsimd(SWDGE): wv bias x10           (0.90MB)
        # One early-critical item leads each queue (wq / x00a / wv); the
        # paired first q-unit consumes one x tile per ~1.2us, matching the
        # per-queue delivery pace, so the PE never gaps after it starts.
        wq_sb = sb.tile([128, I_T, OUT_DIM], BF16, name="wq_sb")
        S_.dma_start(out=wq_sb, in_=wq)
        load_xp(C_, 0, 0, 0)
        wv_sb = sb.tile([128, I_T, OUT_DIM], BF16, name="wv_sb")
        G_.dma_start(out=wv_sb, in_=wv)
        load_xp(S_, 0, 0, 1)
        wk_sb = sb.tile([128, I_T, OUT_DIM], BF16, name="wk_sb")
        C_.dma_start(out=wk_sb, in_=wk)
        bias_sb = sb.tile([128, 4 + OUT_DIM], F32, name="bias_sb")
        G_.dma_start(out=bias_sb, in_=bias)
        bqr_sb = bias_sb[:, 0:O_T]
        bkr_sb = bias_sb[:, O_T:2 * O_T]
        bv_bc = bias_sb[:, 4:4 + OUT_DIM]
        load_xc(C_, 0, 1)
        load_xc(G_, 1, 0)
        load_xc(S_, 1, 1)

        q_sb = [sb.tile([128, O_T, S], BF16, name=f"q{b}") for b in range(B_LOC)]
        k_sb = [sb.tile([128, O_T, S], BF16, name=f"k{b}") for b in range(B_LOC)]
        v_sb = [
            sb.tile([128, S_TILES, VW], BF16, name=f"v{b}")
            for b in range(B_LOC)
        ]
        e_sb = [sb.tile([128, S_TILES, S], BF16, name=f"e{b}") for b in range(B_LOC)]

        # ---- work-unit generators (each unit = one PSUM group + epilogue) ----
        def qk_unit(b, h, w_s, b_s, dst, o):
            ps = ps_pj.tile([128, 512], F32, tag="pj", name="pspj")
            for n, i in enumerate(range(I_T)):
                nc.tensor.matmul(
                    ps,
                    w_s[:, i, o * 128:(o + 1) * 128],
                    xt(b, h, i),
                    start=(n == 0),
                    stop=(n == I_T - 1),
                )
            nc.vector.tensor_scalar_add(
                dst[:, o, h * 512:(h + 1) * 512], ps, b_s[:, o:o + 1],
            )

        def v_unit(b, h, j):
            st = h * (S_TILES // SC) + j
            ps = ps_pj.tile([128, OUT_DIM], F32, tag="pj", name="pspj")
            for i in range(I_T):
                nc.tensor.matmul(
                    ps,
                    xt(b, h, i)[:, j * 128:(j + 1) * 128],
                    wv_sb[:, i, :],
                    start=(i == 0),
                    stop=(i == I_T - 1),
                )
            nc.vector.tensor_add(v_sb[b][:, st, 0:H], ps[:, 0:H],
                                 bv_bc[:, 0:H])
            nc.vector.tensor_add(v_sb[b][:, st, H + 1:VW - 1], ps[:, H:OUT_DIM],
                                 bv_bc[:, H:OUT_DIM])

        def q00_pair():
            # first two q-units with interleaved contractions: one x tile is
            # consumed per ~1.2us (half clock), matching DMA delivery, so the
            # PE never gaps while the rest of the inputs stream in.
            psA = ps_pj.tile([128, 512], F32, tag="pj", name="pspj")
            psB = ps_pj.tile([128, 512], F32, tag="pj", name="pspj")
            for n in range(I_T):
                nc.tensor.matmul(psA, wq_sb[:, n, 0:128], xt(0, 0, n),
                                 start=(n == 0), stop=(n == I_T - 1))
                nc.tensor.matmul(psB, wq_sb[:, n, 128:256], xt(0, 0, n),
                                 start=(n == 0), stop=(n == I_T - 1))
            nc.vector.tensor_scalar_add(q_sb[0][:, 0, 0:512], psA,
                                        bqr_sb[:, 0:1])
            nc.vector.tensor_scalar_add(q_sb[0][:, 1, 0:512], psB,
                                        bqr_sb[:, 1:2])

        def qkv_units(b):
            units = []
            for h in range(SC):
                if b == 0 and h == 0:
                    # first group: q-pair, then v (wv lands early), then k
                    # (wk is 2nd on its queue and lands last of the three)
                    units.append(q00_pair)
                    for j in range(S_TILES // SC):
                        units.append(lambda j=j: v_unit(0, 0, j))
                    for o in range(O_T):
                        units.append(lambda o=o: qk_unit(
                            0, 0, wk_sb, bkr_sb, k_sb[0], o))
                    continue
                for w_s, b_s, dst in (
                    (wq_sb, bqr_sb, q_sb[b]),
                    (wk_sb, bkr_sb, k_sb[b]),
                ):
                    for o in range(O_T):
                        units.append(
                            lambda h=h, o=o, w_s=w_s, b_s=b_s, dst=dst:
                            qk_unit(b, h, w_s, b_s, dst, o))
                for j in range(S_TILES // SC):
                    units.append(lambda h=h, j=j: v_unit(b, h, j))
            return units

        def st_units(b):
            # scores^T tile (sk on partitions, sq chunk on free) + fused exp
            units = []
            for h in range(SC):
                for sk in range(S_TILES):
                    def f(h=h, sk=sk):
                        ps = ps_s.tile([128, 512], F32, tag="s", name="pss")
                        for d in range(O_T):
                            nc.tensor.matmul(
                                ps,
                                k_sb[b][:, d, sk * 128:(sk + 1) * 128],
                                q_sb[b][:, d, h * 512:(h + 1) * 512],
                                start=(d == 0),
                                stop=(d == O_T - 1),
                            )
                        nc.scalar.activation(
                            out=e_sb[b][:, sk, h * 512:(h + 1) * 512],
                            in_=ps,
                            func=mybir.ActivationFunctionType.Exp,
                            scale=SCALE,
                        )
                    units.append(f)
            return units

        OUT_Q = [S_, C_, G_]

        def av_units(b, split_last=False):
            units = []
            for sq in range(S_TILES):
                if split_last and sq == S_TILES - 1:
                    def f(sq=sq):
                        # column-split final unit: half A finishes first and
                        # its divide+store overlaps half B's matmuls. Halves
                        # come from ps_s (free by now) and use dedicated SBUF
                        # tiles so no WAR wait on recycled epilogue slots.
                        psA = ps_s.tile([128, H + 1], F32, tag="s", name="pss")
                        for sk in range(S_TILES):
                            nc.tensor.matmul(
                                psA,
                                e_sb[b][:, sk, sq * 128:(sq + 1) * 128],
                                v_sb[b][:, sk, 0:H + 1],
                                start=(sk == 0),
                                stop=(sk == S_TILES - 1),
                            )
                        psB = ps_s.tile([128, H + 1], F32, tag="s", name="pss")
                        for sk in range(S_TILES):
                            nc.tensor.matmul(
                                psB,
                                e_sb[b][:, sk, sq * 128:(sq + 1) * 128],
                                v_sb[b][:, sk, H + 1:VW],
                                start=(sk == 0),
                                stop=(sk == S_TILES - 1),
                            )
                        recA = sb.tile([128, 1], F32, name="lrecA")
                        nc.vector.reciprocal(recA, psA[:, H:H + 1])
                        otA = sb.tile([128, H], F32, name="lotA")
                        nc.vector.tensor_scalar_mul(otA, psA[:, 0:H], recA)
                        nc.sync.dma_start(
                            out=out[b, sq * 128:(sq + 1) * 128, 0:H], in_=otA)
                        recB = sb.tile([128, 1], F32, name="lrecB")
                        nc.vector.reciprocal(recB, psB[:, H:H + 1])
                        otB = sb.tile([128, H], F32, name="lotB")
                        nc.vector.tensor_scalar_mul(otB[:, 0:H // 2],
                                                    psB[:, 0:H // 2], recB)
                        nc.scalar.dma_start(
                            out=out[b, sq * 128:(sq + 1) * 128, H:H + H // 2],
                            in_=otB[:, 0:H // 2])
                        nc.vector.tensor_scalar_mul(otB[:, H // 2:H],
                                                    psB[:, H // 2:H], recB)
                        nc.sync.dma_start(
                            out=out[b, sq * 128:(sq + 1) * 128, H + H // 2:OUT_DIM],
                            in_=otB[:, H // 2:H])
                    units.append(f)
                    continue

                def f(sq=sq):
                    ps = ps_av.tile([128, VW], F32, tag="av", name="psav")
                    for sk in range(S_TILES):
                        nc.tensor.matmul(
                            ps,
                            e_sb[b][:, sk, sq * 128:(sq + 1) * 128],
                            v_sb[b][:, sk, :],
                            start=(sk == 0),
                            stop=(sk == S_TILES - 1),
                        )
                    rec = small.tile([128, 1], F32, tag="rec", name="rec")
                    nc.vector.reciprocal(rec, ps[:, H:H + 1])
                    ot = small.tile([128, OUT_DIM], F32, tag="ot", name="ot")
                    nc.vector.tensor_scalar_mul(ot[:, 0:H], ps[:, 0:H], rec)
                    nc.vector.tensor_scalar_mul(ot[:, H:OUT_DIM],
                                                ps[:, H + 1:VW - 1], rec)
                    OUT_Q[(b * S_TILES + sq) % 3].dma_start(
                        out=out[b, sq * 128:(sq + 1) * 128, :], in_=ot
                    )
                units.append(f)
            return units

        # ---- PE warm-up: keep TensorE busy while input DMAs land so the
        # HAM clock ramp starts early and never resets; sized to hand off
        # to the first real matmul with no gap ----
        junk = sb.tile([128, 128], BF16, name="junk")
        nc.vector.memset(junk, 0.0)
        for _ in range(N_WARM):
            wps = ps_av.tile([128, 128], F32, tag="av", name="warmps")
            for _k in range(2):
                nc.tensor.matmul(wps, junk, junk, start=(_k == 0),
                                 stop=(_k == 1))

        # ones-planes of v (cols H and VW-1), never overwritten by v units
        for b in range(B_LOC):
            nc.vector.memset(v_sb[b][:, :, H:H + 1], 1.0)
            nc.vector.memset(v_sb[b][:, :, VW - 1:VW], 1.0)

        # ---- emission: st0 (no DMA deps) runs before qkv1 so the b=1
        # x tiles get ~7us more delivery slack; PE executes in-order, so a
        # single late DMA dep would stall it and down-throttle the clock ----
        for u in qkv_units(0):
            u()
        for a in st_units(0):
            a()
        for u in qkv_units(1):
            u()
        st1 = st_units(1)
        av0 = av_units(0)
        for idx in range(S_TILES):
            st1[2 * idx]()
            st1[2 * idx + 1]()
            av0[idx]()
        for u in av_units(1, split_last=True):
            u()

    nc.compile()
    return nc


_CACHE = {}


def _get_nc():
    if "nc" not in _CACHE:
        _CACHE["nc"] = _build()
    return _CACHE["nc"]


def _rope_fold(w, bvec):
    """Fold the (position-independent) RoPE rotation into weights/bias.

    Mirrors the reference: inv_freq over arange(0, OUT_DIM, 2)/OUT_DIM,
    angle = 2*S*inv_freq, pairs (2j, 2j+1) rotated by angle_j.
    Computed in float32 to track the reference's f32 arithmetic.
    """
    exps = np.arange(0, OUT_DIM, 2, dtype=np.float32) / np.float32(OUT_DIM)
    inv = (np.float32(1.0) / np.power(np.float32(THETA), exps)).astype(np.float32)
    ang = (np.float32(2.0 * S) * inv).astype(np.float32)
    cos = np.cos(ang).astype(np.float32)
    sin = np.sin(ang).astype(np.float32)

    w2 = w.reshape(OUT_DIM // 2, 2, IN_DIM)
    wr = np.empty_like(w2)
    wr[:, 0] = cos[:, None] * w2[:, 0] - sin[:, None] * w2[:, 1]
    wr[:, 1] = sin[:, None] * w2[:, 0] + cos[:, None] * w2[:, 1]
    b2 = bvec.reshape(OUT_DIM // 2, 2)
    br = np.empty_like(b2)
    br[:, 0] = cos * b2[:, 0] - sin * b2[:, 1]
    br[:, 1] = sin * b2[:, 0] + cos * b2[:, 1]
    return wr.reshape(OUT_DIM, IN_DIM), br.reshape(OUT_DIM)


def _pack_w(w_rot, bf16):
    """[OUT_DIM, IN_DIM] weight -> [128, I_T, OUT_DIM] bf16 SBUF tile layout."""
    wt = np.ascontiguousarray(w_rot.T)                  # [IN_DIM, OUT_DIM]
    return np.ascontiguousarray(
        wt.reshape(I_T, 128, OUT_DIM).transpose(1, 0, 2)
    ).astype(bf16)


def _prep_inputs(x, wq, bq, wk, bk, wv, bv):
    import ml_dtypes
    bf16 = ml_dtypes.bfloat16
    x = np.asarray(x, dtype=np.float32)
    wq_r, bq_r = _rope_fold(np.asarray(wq, np.float32), np.asarray(bq, np.float32))
    wk_r, bk_r = _rope_fold(np.asarray(wk, np.float32), np.asarray(bk, np.float32))
    wv = np.asarray(wv, np.float32)
    bv = np.asarray(bv, np.float32)

    bias = np.empty((128, 4 + OUT_DIM), np.float32)
    bias[:, 0:O_T] = bq_r.reshape(O_T, 128).T
    bias[:, O_T:2 * O_T] = bk_r.reshape(O_T, 128).T
    bias[:, 4:] = bv[None, :]
    shared = {
        "wq": _pack_w(wq_r, bf16),
        "wk": _pack_w(wk_r, bf16),
        "wv": _pack_w(wv, bf16),
        "bias": np.ascontiguousarray(bias),
    }
    in_maps = []
    for c in range(N_CORES):
        shard = x[c * B_LOC:(c + 1) * B_LOC]            # [B_LOC, S, IN_DIM]
        # xh[b, h, p, i, s] = shard[b, h*512+s, i*128+p]
        xh = np.ascontiguousarray(
            shard.transpose(0, 2, 1)                     # [b, IN_DIM, S]
            .reshape(B_LOC, I_T, 128, SC, 512)
            .transpose(0, 3, 2, 1, 4)
        ).astype(bf16)
        in_maps.append({"xh": xh, **shared})
    return in_maps


def _execute(in_maps, trace=False, tmpdir=None):
    nc = _get_nc()
    return run_bass_kernel_spmd(
        nc, in_maps, core_ids=list(range(N_CORES)), trace=trace, tmpdir=tmpdir
    )


def kernel(x, wq, bq, wk, bk, wv, bv):
    in_maps = _prep_inputs(x, wq, bq, wk, bk, wv, bv)
    res = _execute(in_maps)
    return np.concatenate(
        [np.asarray(res.results[i]["out"]) for i in range(N_CORES)], axis=0
    ).astype(np.float32)
